# revision 8
# baseline (speedup 1.0000x reference)
"""Trainium2 Bass kernel for nn_ChannelAttention (channel attention + residual + layernorm).

Math: q = x@Wq.T+bq (per-head); k,v from channel_embeddings only (19 channels).
scores[t,c,h] = q_h(t)·k_h(c)/sqrt(dh)  -> softmax over c -> attended -> @Wo.T+bo
y = LayerNorm(x + out) * gamma + beta.

Because k/v depend only on the 19 channel embeddings, fold everything into two
small host-precomputed matrices:
  A[j=(h,c), d]  : scores = x @ A.T  (+ sbias row via K=1 matmul)   [77 x 512]
                   row 76 of A is all-ones -> col 76 of s = sum_d x (for LN mean)
  U[j, d]        : attnout = w @ U, row 76 = bo (w col 76 == 1)     [77 x 512]
  Urs[j]         : row sums of U -> sum_d attnout via small matmul
On-chip per 128-token tile: transpose x (PE), s = xT.T@A chunks (PE), exp (ACT),
per-head sums (DVE reduce), normalize (DVE), transpose w (PE), attnout+residual
via identity matmul (PE, accumulated in PSUM), layernorm stats + affine
(ACT/DVE), gamma/beta (DVE/GPSIMD), casted DMA in/out (SWDGE).

Data-parallel over tokens across 8 NeuronCores; constants replicated.
"""

import sys

sys.path.insert(0, "/opt/trn_rl_repo")

import numpy as np
import ml_dtypes

import concourse.bass as bass
import concourse.bacc as bacc
import concourse.mybir as mybir
import concourse.tile as tile
from concourse.bass_utils import run_bass_kernel_spmd

BF16 = ml_dtypes.bfloat16

# Problem dims
B, P, C, D, H = 32, 2048, 19, 512, 4
DH = D // H  # 128
EPS = 1e-5
N_CORES = 8
TOK_TOTAL = B * P           # 65536
TOK_CORE = TOK_TOTAL // N_CORES  # 8192

J = C * H + 1               # 77: 76 score cols + ones row (x-sum)
NS = J + 1                  # 78: + asum col in score PSUM tile

# Per-core tiling
SUB = 8                     # 128-token sub-tiles per block
BLK = 128 * SUB             # 1024 tokens per DMA block
NBLK_FULL = TOK_CORE // BLK  # 8

# Const buffer column layout (bf16, [128, CB_COLS])
_AT0 = 0                     # A.T chunks [128, 4, 77]
_ID0 = _AT0 + 4 * J          # identity [128, 128]
_U0 = _ID0 + 128             # U rows (partitions 0..76) [77, 512]
_URS0 = _U0 + D              # U row sums [77, 1]
_SB0 = _URS0 + 1             # sbias row [1, 77]
_ONE0 = _SB0 + J             # ones row [1, 128]
_GAM0 = _ONE0 + 128          # gamma broadcast [128, 512]
_BET0 = _GAM0 + D            # beta broadcast [128, 512]
CB_COLS = _BET0 + D

FP32 = mybir.dt.float32
BF = mybir.dt.bfloat16
AF = mybir.ActivationFunctionType
OP = mybir.AluOpType


def _precompute_consts(channel_embeddings, Wq, bq, Wk, bk, Wv, bv, Wo, bo,
                       gamma, beta):
    """Fold attention weights into A [J,512], sbias [J], U [J,512], Urs [J]."""
    f8 = np.float64
    ce = channel_embeddings.astype(f8)
    k = (ce @ Wk.astype(f8).T + bk.astype(f8)).reshape(C, H, DH)
    v = (ce @ Wv.astype(f8).T + bv.astype(f8)).reshape(C, H, DH)
    scale = 1.0 / np.sqrt(DH)

    A = np.zeros((J, D), f8)
    sbias = np.zeros((J,), f8)
    U = np.zeros((J, D), f8)
    Wq8, bq8, Wo8, bo8 = (Wq.astype(f8), bq.astype(f8), Wo.astype(f8),
                          bo.astype(f8))
    for h in range(H):
        hs = slice(h * DH, (h + 1) * DH)
        for c in range(C):
            j = h * C + c
            A[j] = (k[c, h] @ Wq8[hs, :]) * scale
            sbias[j] = (bq8[hs] @ k[c, h]) * scale
            U[j] = Wo8[:, hs] @ v[c, h]
    A[J - 1] = 1.0          # x-sum row
    sbias[J - 1] = 0.0
    U[J - 1] = bo8          # bo rides on the constant-1 w column
    Urs = U.sum(axis=1)

    cb = np.zeros((128, CB_COLS), f8)
    # A.T chunks: cb[p, k*J + j] = A[j, 128k+p]
    for kk in range(4):
        cb[:, _AT0 + kk * J:_AT0 + (kk + 1) * J] = A[:, kk * 128:(kk + 1) * 128].T
    cb[:, _ID0:_ID0 + 128] = np.eye(128)
    cb[:J, _U0:_U0 + D] = U
    cb[:J, _URS0] = Urs
    cb[0, _SB0:_SB0 + J] = sbias
    cb[0, _ONE0:_ONE0 + 128] = 1.0
    cb[:, _GAM0:_GAM0 + D] = gamma.astype(f8)[None, :]
    cb[:, _BET0:_BET0 + D] = beta.astype(f8)[None, :]
    return cb.astype(BF16)


def build_nc(nblk=NBLK_FULL, reps=1):
    """Build + compile the per-core SPMD graph for nblk blocks of 1024 tokens.

    reps>1 repeats the whole pass (same input/output regions) — used only for
    amortized on-device timing; results are identical to reps=1.
    """
    tok = nblk * BLK
    nc = bacc.Bacc("TRN2", target_bir_lowering=False, debug=False,
                   enable_asserts=False, num_devices=N_CORES)
    x_d = nc.dram_tensor("x", [tok, D], FP32, kind="ExternalInput")
    cb_d = nc.dram_tensor("cb", [128, CB_COLS], BF, kind="ExternalInput")
    out_d = nc.dram_tensor("out", [tok, D], FP32, kind="ExternalOutput")

    x_v = x_d.ap().rearrange("(nb a p) d -> nb p a d", p=128, a=SUB)
    o_v = out_d.ap().rearrange("(nb a p) d -> nb p a d", p=128, a=SUB)

    with tile.TileContext(nc) as tc:
        with (
            tc.tile_pool(name="cpool", bufs=1) as cpool,
            tc.tile_pool(name="xpool", bufs=2) as xpool,
            tc.tile_pool(name="ypool", bufs=2) as ypool,
            tc.tile_pool(name="work", bufs=3) as work,
            tc.tile_pool(name="scr", bufs=1) as scr,
            tc.tile_pool(name="stat", bufs=3) as stat,
            tc.tile_pool(name="ps_xT", bufs=2, space="PSUM") as ps_xT,
            tc.tile_pool(name="ps_s", bufs=2, space="PSUM") as ps_s,
            tc.tile_pool(name="ps_h", bufs=2, space="PSUM") as ps_h,
            tc.tile_pool(name="ps_wT", bufs=2, space="PSUM") as ps_wT,
        ):
            cb = cpool.tile([128, CB_COLS], BF)
            nc.sync.dma_start(out=cb[:], in_=cb_d.ap())
            AT = cb[:, _AT0:_ID0].rearrange("p (k j) -> p k j", j=J)
            ID = cb[:, _ID0:_ID0 + 128]
            U = cb[0:J, _U0:_U0 + D]
            URS = cb[0:J, _URS0:_URS0 + 1]
            SBR = cb[0:1, _SB0:_SB0 + J]
            ONE = cb[0:1, _ONE0:_ONE0 + 128]
            GAM = cb[:, _GAM0:_GAM0 + D]
            BET = cb[:, _BET0:_BET0 + D]

            sq_scr = scr.tile([128, D], BF)  # write-only Square scratch
            eps_t = cpool.tile([128, 1], FP32)
            nc.gpsimd.memset(eps_t[:], EPS)

            for nb_r in range(nblk * reps):
                nb = nb_r % nblk
                x_blk = xpool.tile([128, SUB, D], BF, tag="xblk")
                nc.gpsimd.dma_start(out=x_blk[:], in_=x_v[nb])  # f32->bf16 cast
                y_blk = ypool.tile([128, SUB, D], BF, tag="yblk")

                for a in range(SUB):
                    xa = x_blk[:, a, :]
                    # -- x transpose (PE), 4 chunks of [128,128] --
                    xT_ps = ps_xT.tile([128, 4, 128], BF, tag="xT")
                    for kk in range(4):
                        nc.tensor.transpose(
                            xT_ps[:, kk, :], xa[:, kk * 128:(kk + 1) * 128], ID)
                    xTs = work.tile([128, 4, 128], BF, tag="xTs")
                    nc.scalar.copy(xTs[:, 0:2, :], xT_ps[:, 0:2, :])
                    nc.vector.tensor_copy(xTs[:, 2:4, :], xT_ps[:, 2:4, :])

                    # -- scores: s[t, j] accumulated over 4 K-chunks + bias row
                    s_ps = ps_s.tile([128, J], FP32, tag="s")
                    for kk in range(4):
                        nc.tensor.matmul(s_ps[:, 0:J], xTs[:, kk, :],
                                         AT[:, kk, :],
                                         start=(kk == 0), stop=False)
                    nc.tensor.matmul(s_ps[:, 0:J], ONE, SBR,
                                     start=False, stop=True)

                    # -- softmax over 19 channels within each of 4 heads --
                    w_b = work.tile([128, J], BF, tag="w")
                    nc.scalar.activation(w_b[:, 0:J - 1], s_ps[:, 0:J - 1],
                                         AF.Exp)
                    nc.gpsimd.memset(w_b[:, J - 1:J], 1.0)
                    Z4 = stat.tile([128, 4], FP32, tag="Z4")
                    nc.vector.tensor_reduce(
                        Z4[:], w_b[:, 0:J - 1].rearrange("p (h c) -> p h c", c=C),
                        axis=mybir.AxisListType.X, op=OP.add)
                    iZ4 = stat.tile([128, 4], FP32, tag="iZ4")
                    nc.vector.reciprocal(iZ4[:], Z4[:])
                    for h in range(H):
                        nc.vector.tensor_scalar_mul(
                            w_b[:, h * C:(h + 1) * C], w_b[:, h * C:(h + 1) * C],
                            iZ4[:, h:h + 1])

                    # -- w transpose (PE) --
                    wT_ps = ps_wT.tile([J, 128], BF, tag="wT")
                    nc.tensor.transpose(wT_ps[:], w_b[:], ID)
                    wTs = work.tile([J, 128], BF, tag="wTs")
                    nc.vector.tensor_copy(wTs[:], wT_ps[:])

                    # -- attnout + residual: h = w@U + I@x  (PSUM accumulate) --
                    h_ps = ps_h.tile([128, D], FP32, tag="h")
                    nc.tensor.matmul(h_ps[:], wTs[:], U, start=True, stop=False)
                    nc.tensor.matmul(h_ps[:], ID, xa, start=False, stop=True)
                    # asum = sum_d attnout (+bo) via U row sums, accumulated
                    # onto the x-sum column -> col J-1 becomes sum_d h
                    nc.tensor.matmul(s_ps[:, J - 1:J], wTs[:], URS,
                                     start=False, stop=True,
                                     skip_group_check=True)

                    # -- layernorm stats --
                    hs = s_ps[:, J - 1:J]
                    sumsq = stat.tile([128, 1], FP32, tag="sumsq")
                    nc.scalar.activation(sq_scr[:], h_ps[:], AF.Square,
                                         accum_out=sumsq[:])
                    nm = stat.tile([128, 1], FP32, tag="nm")
                    nc.vector.tensor_scalar_mul(nm[:], hs, -1.0 / D)
                    m2 = stat.tile([128, 1], FP32, tag="m2")
                    nc.vector.tensor_mul(m2[:], nm[:], nm[:])
                    var = stat.tile([128, 1], FP32, tag="var")
                    nc.vector.tensor_scalar(var[:], sumsq[:], 1.0 / D, m2[:],
                                            op0=OP.mult, op1=OP.subtract)
                    sd = stat.tile([128, 1], FP32, tag="sd")
                    nc.scalar.activation(sd[:], var[:], AF.Sqrt, bias=eps_t[:])
                    rstd = stat.tile([128, 1], FP32, tag="rstd")
                    nc.vector.reciprocal(rstd[:], sd[:])
                    nmr = stat.tile([128, 1], FP32, tag="nmr")
                    nc.vector.tensor_mul(nmr[:], nm[:], rstd[:])

                    # -- z = (h - mean) * rstd ; y = z*gamma + beta --
                    z_b = work.tile([128, D], BF, tag="z")
                    nc.scalar.activation(z_b[:], h_ps[:], AF.Identity,
                                         bias=nmr[:], scale=rstd[:])
                    y1 = work.tile([128, D], BF, tag="y1")
                    nc.vector.tensor_mul(y1[:], z_b[:], GAM)
                    nc.gpsimd.tensor_add(y_blk[:, a, :], y1[:], BET)

                nc.gpsimd.dma_start(out=o_v[nb], in_=y_blk[:])  # bf16->f32

    nc.compile()
    return nc


_NC_CACHE = {}


def _get_nc(nblk):
    if nblk not in _NC_CACHE:
        _NC_CACHE[nblk] = build_nc(nblk)
    return _NC_CACHE[nblk]


def kernel(x, channel_embeddings, Wq, bq, Wk, bk, Wv, bv, Wo, bo, gamma, beta):
    x = np.asarray(x, np.float32)
    cb = _precompute_consts(
        np.asarray(channel_embeddings), np.asarray(Wq), np.asarray(bq),
        np.asarray(Wk), np.asarray(bk), np.asarray(Wv), np.asarray(bv),
        np.asarray(Wo), np.asarray(bo), np.asarray(gamma), np.asarray(beta))

    xf = np.ascontiguousarray(x.reshape(TOK_TOTAL, D))
    nc = _get_nc(NBLK_FULL)
    in_maps = [
        {"x": xf[i * TOK_CORE:(i + 1) * TOK_CORE], "cb": cb}
        for i in range(N_CORES)
    ]
    res = run_bass_kernel_spmd(nc, in_maps, core_ids=list(range(N_CORES)))
    out = np.concatenate([res.results[i]["out"] for i in range(N_CORES)], axis=0)
    return out.reshape(B, P, D).astype(np.float32)
